# revision 1
# baseline (speedup 1.0000x reference)
"""LMU (Legendre Memory Unit) RNN kernel for Trainium2, 8 NeuronCores.

Strategy
--------
Data-parallel over batch: each of the 8 cores runs B_local = 16 sequences
through the full T=784-step recurrence; outputs are concatenated on host.

Per-step math is algebraically folded into a single affine map.  With
u_t = e_x x_t + h_t e_h^T + m_t e_m^T, Ad = I + AT, w = W_m @ BT:

    m_{t+1} = (Ad + BT e_m) m_t + (BT e_h) h_t + (BT e_x) x_t
    pre_h   = (W_h + w e_h) h_t + (W_m Ad + w e_m) m_t + (W_x + w e_x) x_t
    h_{t+1} = tanh(pre_h)

so each step is one matmul  out[16, 1536] = s[16, 1537] @ M_big.T  plus a
tanh on the h columns.  M_big is precomputed on host in float64 -> fp16.

On-chip schedule (PE column tiling).  The per-core batch is only 16, so
each matmul's stationary operand (a state K-tile, [128, 16]) occupies
just 16 of the PE array's 128 columns.  The 1536 output features are
split into 4 strips of 384 issued to the four 32-column groups of the PE
array (tile_position=(0, 32j)); the 4 weight streams run concurrently,
cutting the per-step matmul span ~4x (measured round pitch ~163ns =
384 cols @ 2.4 GHz, stream-bound).

The four PSUM strips live at partitions {0-15, 32-47, 64-79, 96-111} of
one [128, 384] PSUM tile, so the step output is re-transposed with just
3 full-width (128,128) PE transposes (instead of 12 thin ones).  Output
features are permuted on host so each strip is 256 h columns + 128 m
columns; after transposing, chunk 0/1 are pure h (tanh on ACT at full
lane utilization) and chunk 2 is pure m (DVE copy).

The per-step x contribution (rank-1 per strip) is ONE K=4 matmul: lhsT
row k holds x_t spread at columns 32k:32k+16 (host-built), rhs row k is
strip k's xw.  Its full-width start=True write also keeps the
never-again-touched psum partitions (16:32 of each 32-group) zeroed.

Feature order bookkeeping (host-side permutations):
  out-slot (j, c):   OP[j*384+c] = 256j+c          (c < 256, h)
                                 = 1024+128j+c-256  (c >= 256, m)
  state tile (t, j): IP[(t*4+j)*128+c] = 256j+128t+c (t<2, h)
                                       = 1024+128j+c (t=2, m)
The state tile (t, j) is st[:, 128t+32j : +16] after transpose, and
weight block r = t*4+j rows follow IP so state/weights stay aligned.

Empirical notes (HW traces):
- PSUM dependency tracking is tile-granular: column-splitting rounds to
  unblock the first cast early does not help (v5 regression), and the
  three transposes + tanh/copy serialize via the shared tp tile.
- unroll=112 and restructured transposes (v6/v7) perturbed the PE clock
  / scheduler into ~183ns round pitch; this arrangement reaches ~163ns.
"""

import numpy as np

import concourse.bass as bass
import concourse.mybir as mybir
import concourse.tile as tile
from concourse import bacc
from concourse.bass import ds, ts
from concourse.bass_utils import run_bass_kernel_spmd
from concourse.masks import make_identity

T, B, UNITS, ORDER = 784, 128, 1024, 512
NCORES = 8
BL = B // NCORES          # 16 sequences per core
SOUT = UNITS + ORDER      # 1536 output features [pre_h; m']
KT = 12                   # state K-tiles of 128
NSTRIP = 4                # PE column groups
NW = SOUT // NSTRIP       # 384 output columns per strip

FP16 = mybir.dt.float16
FP32 = mybir.dt.float32


def _perms():
    OP = np.empty(SOUT, np.int64)
    for j in range(NSTRIP):
        for c in range(NW):
            OP[j * NW + c] = 256 * j + c if c < 256 else 1024 + 128 * j + (c - 256)
    IP = np.empty(SOUT, np.int64)
    for t in range(3):
        for j in range(4):
            for c in range(128):
                IP[(t * 4 + j) * 128 + c] = (
                    256 * j + 128 * t + c if t < 2 else 1024 + 128 * j + c
                )
    return OP, IP


def _build_weights(e_x, e_h, e_m, W_x, W_h, W_m, AT, BT):
    """Host-side fold into M_big (float64), then permute rows/cols."""
    f = np.float64
    e_x, e_h, e_m = e_x.astype(f), e_h.astype(f), e_m.astype(f)
    W_x, W_h, W_m = W_x.astype(f), W_h.astype(f), W_m.astype(f)
    AT, BT = AT.astype(f), BT.astype(f)
    Ad = np.eye(ORDER) + AT
    w = W_m @ BT                                   # (U, 1)
    top = np.concatenate([W_h + w @ e_h, W_m @ Ad + w @ e_m, W_x + w * e_x], axis=1)
    bot = np.concatenate([BT @ e_h, Ad + BT @ e_m, BT * e_x], axis=1)
    M_big = np.concatenate([top, bot], axis=0)     # (1536 out, 1537 in)
    OP, IP = _perms()
    Wstate = M_big[np.ix_(OP, IP)].T               # (in-perm, out-slot)
    xw = M_big[OP, SOUT]                           # x column, out-slot order
    return Wstate.astype(np.float16), xw.reshape(1, SOUT).astype(np.float16)


def _build_nc(t_steps=T, unroll=56):
    assert t_steps % unroll == 0 and unroll % 2 == 0
    iters = t_steps // unroll
    nc = bacc.Bacc("TRN2", target_bir_lowering=False, num_devices=NCORES)

    w_dram = nc.dram_tensor("wts", [KT * 128, SOUT], FP16, kind="ExternalInput")
    xw_dram = nc.dram_tensor("xw", [NSTRIP, NW], FP16, kind="ExternalInput")
    wd_dram = nc.dram_tensor("wd", [UNITS + 1, 10], FP16, kind="ExternalInput")
    # one body of zero padding at the end: body i prefetches slice i+1
    x_dram = nc.dram_tensor(
        "xs", [NSTRIP, (t_steps + unroll) * 128], FP16, kind="ExternalInput"
    )
    out_dram = nc.dram_tensor("out", [BL, 10], FP32, kind="ExternalOutput")

    TANH = mybir.ActivationFunctionType.Tanh

    with tile.TileContext(nc) as tc:
        with (
            tc.tile_pool(name="const", bufs=1) as cpool,
            tc.tile_pool(name="state", bufs=1) as spool,
            tc.tile_pool(name="work", bufs=2) as wpool,
            tc.tile_pool(name="psum", bufs=1, space="PSUM") as ppool,
        ):
            # ---- persistent SBUF ----
            w_sb = cpool.tile([128, KT * SOUT], FP16, tag="w_sb")
            for r in range(KT):
                nc.sync.dma_start(w_sb[:, ts(r, SOUT)], w_dram[ts(r, 128), :])
            xw_sb = cpool.tile([NSTRIP, NW], FP16, tag="xw_sb")
            nc.sync.dma_start(xw_sb[:, :], xw_dram[:, :])
            wd_sb = cpool.tile([128, 8 * 10], FP16, tag="wd_sb")
            for r in range(8):
                nc.sync.dma_start(wd_sb[:, ts(r, 10)], wd_dram[ts(r, 128), :])
            bias_sb = cpool.tile([1, 10], FP16, tag="bias_sb")
            nc.sync.dma_start(bias_sb[:, :], wd_dram[1024:1025, :])
            ones_sb = cpool.tile([1, BL], FP16, tag="ones_sb")
            nc.vector.memset(ones_sb[:, :], 1.0)
            ident = cpool.tile([128, 128], FP16, tag="ident")
            make_identity(nc, ident[:, :])

            stA = spool.tile([128, 3 * 128], FP16, tag="stA")
            stB = spool.tile([128, 3 * 128], FP16, tag="stB")
            sfA = spool.tile([128, NW], FP16, tag="sfA")
            sfB = spool.tile([128, NW], FP16, tag="sfB")
            for t_ in (stA, stB, sfA, sfB):
                nc.vector.memset(t_[:, :], 0.0)
            x_stage = spool.tile([NSTRIP, unroll * 128], FP16, tag="x_stage")
            # prologue: body 0's x slice
            nc.sync.dma_start(x_stage[:, :], x_dram[:, 0 : unroll * 128])

            psA = ppool.tile([128, NW], FP32, tag="psA")
            psB = ppool.tile([128, NW], FP32, tag="psB")
            # one psum tile PER transposed chunk: dependency tracking is
            # tile-granular, so a shared tile falsely serializes
            # T_2 -> tanh_1 -> T_1 -> tanh_0 and stalls the PE FIFO
            # between the next step's first rounds
            tpA = [
                ppool.tile([128, 128], FP16, tag=f"tpA{t}", name=f"tpA{t}")
                for t in range(3)
            ]
            tpB = [
                ppool.tile([128, 128], FP16, tag=f"tpB{t}", name=f"tpB{t}")
                for t in range(3)
            ]

            def xround(ps, x_ap):
                # all four strips' rank-1 x contribution in ONE K=4 matmul
                nc.tensor.matmul(
                    ps[:, :], x_ap, xw_sb[:, :], start=True, stop=False,
                )

            def step(src, dst, ps, ps_next, tp, sf, x_next):
                # next step's x round first: its deps (x_stage, ps_next) are
                # ready early, so the scheduler can slot it into the PE gap
                # while this step waits on the previous tanh
                if x_next is not None:
                    xround(ps_next, x_next)
                # 12 K-rounds; within a round the 4 strip matmuls go to the
                # four PE column groups and stream concurrently.  The last
                # round is split column-wise (cols 128:NW first, 0:128
                # last-but-short); empirically this exact arrangement (with
                # unroll=56) schedules ~15% faster than the uniform one.
                for r in range(KT):
                    lhsT = src[:, ds(128 * (r // 4) + 32 * (r % 4), BL)]
                    if r < KT - 1:
                        for j in range(NSTRIP):
                            nc.tensor.matmul(
                                ps[32 * j : 32 * j + BL, :],
                                lhsT,
                                w_sb[:, ds(r * SOUT + j * NW, NW)],
                                start=False,
                                stop=False,
                                tile_position=(0, 32 * j),
                            )
                    else:
                        for j in range(NSTRIP):
                            nc.tensor.matmul(
                                ps[32 * j : 32 * j + BL, 128:NW],
                                lhsT,
                                w_sb[:, ds(r * SOUT + j * NW + 128, NW - 128)],
                                start=False,
                                stop=True,
                                tile_position=(0, 32 * j),
                            )
                        for j in range(NSTRIP):
                            nc.tensor.matmul(
                                ps[32 * j : 32 * j + BL, 0:128],
                                lhsT,
                                w_sb[:, ds(r * SOUT + j * NW, 128)],
                                start=False,
                                stop=True,
                                tile_position=(0, 32 * j),
                            )
                # raw psum -> sbuf fp16 (pre-activation); chunk 0 first so
                # the critical chain to the next step's rounds unblocks asap
                nc.vector.tensor_copy(sf[:, 0:128], ps[:, 0:128])
                nc.vector.tensor_copy(sf[:, 128:NW], ps[:, 128:NW])
                nc.tensor.transpose(tp[0][:, :], sf[:, 0:128], ident[:, :])
                # finalize state: tanh on h chunks, copy on m chunk
                nc.scalar.activation(dst[:, 0:128], tp[0][:, :], TANH)
                nc.tensor.transpose(tp[1][:, :], sf[:, 128:256], ident[:, :])
                nc.scalar.activation(dst[:, 128:256], tp[1][:, :], TANH)
                nc.tensor.transpose(tp[2][:, :], sf[:, 256:384], ident[:, :])
                nc.vector.tensor_copy(dst[:, 256:384], tp[2][:, :])

            with tc.For_i(0, iters, hint_engines=(mybir.EngineType.PE,)) as i:
                xround(psA, x_stage[:, 0:128])
                for u in range(unroll):
                    src, dst = (stA, stB) if u % 2 == 0 else (stB, stA)
                    ps, psn = (psA, psB) if u % 2 == 0 else (psB, psA)
                    tp = tpA if u % 2 == 0 else tpB
                    sf = sfA if u % 2 == 0 else sfB
                    x_next = (
                        x_stage[:, ts(u + 1, 128)] if u < unroll - 1 else None
                    )
                    step(src, dst, ps, psn, tp, sf, x_next)
                # prefetch next body's x while the last steps still run;
                # the WAR on x_stage (last read at step unroll-2) orders it
                nc.sync.dma_start(
                    x_stage[:, :],
                    x_dram[:, ds((i + 1) * (unroll * 128), unroll * 128)],
                )

            # ---- epilogue: logits = h W_d^T + b ; softmax ----
            # final state is in stA (t_steps even); h = chunks t=0,1
            ps_l = psA[0:BL, 0:10]   # reuse psA's bank; loop is done
            for kt in range(8):
                nc.tensor.matmul(
                    ps_l,
                    stA[:, ds(128 * (kt // 4) + 32 * (kt % 4), BL)],
                    wd_sb[:, ts(kt, 10)],
                    start=(kt == 0),
                    stop=False,
                )
            nc.tensor.matmul(
                ps_l, ones_sb[:, :], bias_sb[:, :], start=False, stop=True
            )
            sm = wpool.tile([BL, 10], FP32, tag="sm")
            nc.scalar.activation(sm[:, :], ps_l, mybir.ActivationFunctionType.Exp)
            ssum = wpool.tile([BL, 1], FP32, tag="ssum")
            nc.vector.reduce_sum(ssum[:, :], sm[:, :], axis=mybir.AxisListType.X)
            srec = wpool.tile([BL, 1], FP32, tag="srec")
            nc.vector.reciprocal(srec[:, :], ssum[:, :])
            nc.vector.tensor_scalar_mul(sm[:, :], sm[:, :], srec[:, :])
            nc.sync.dma_start(out_dram[:, :], sm[:, :])

    nc.compile()
    return nc


_NC_CACHE = {}


def _get_nc(t_steps=T, unroll=56):
    key = (t_steps, unroll)
    if key not in _NC_CACHE:
        _NC_CACHE[key] = _build_nc(t_steps, unroll)
    return _NC_CACHE[key]


def kernel(inputs, e_x, e_h, e_m, W_x, W_h, W_m, AT, BT, W_dense, b_dense,
           _t_steps=T, _unroll=56, _trace=False):
    inputs = np.asarray(inputs, np.float32)
    args = [np.asarray(a, np.float32)
            for a in (e_x, e_h, e_m, W_x, W_h, W_m, AT, BT, W_dense, b_dense)]
    e_x, e_h, e_m, W_x, W_h, W_m, AT, BT, W_dense, b_dense = args

    wts, xw = _build_weights(e_x, e_h, e_m, W_x, W_h, W_m, AT, BT)
    xw = np.ascontiguousarray(xw.reshape(NSTRIP, NW))
    _, IP = _perms()
    wd = np.zeros((UNITS + 1, 10), np.float16)
    wd[:UNITS, :] = W_dense.T[IP[:UNITS], :].astype(np.float16)
    wd[UNITS, :] = b_dense.astype(np.float16)

    x = inputs[:_t_steps, :, 0].astype(np.float16)        # (T, B)
    nc = _get_nc(_t_steps, _unroll)
    in_maps = []
    for c in range(NCORES):
        xc = x[:, c * BL:(c + 1) * BL]                    # (T, BL)
        # x-spread for the single K=4 x matmul: row k carries x_t at
        # columns t*128 + 32k + b; one body of zero padding at the end
        xs4 = np.zeros((NSTRIP, _t_steps + _unroll, 128), np.float16)
        for k in range(NSTRIP):
            xs4[k, :_t_steps, 32 * k : 32 * k + BL] = xc
        xs4 = np.ascontiguousarray(
            xs4.reshape(NSTRIP, (_t_steps + _unroll) * 128)
        )
        in_maps.append({"wts": wts, "xw": xw, "wd": wd, "xs": xs4})

    res = run_bass_kernel_spmd(
        nc, in_maps, core_ids=list(range(NCORES)), trace=_trace
    )
    out = np.concatenate([res.results[c]["out"] for c in range(NCORES)], axis=0)
    kernel.last_results = res
    return out.astype(np.float32)



# revision 4
# speedup vs baseline: 1.2239x; 1.2239x over previous
"""LMU (Legendre Memory Unit) RNN kernel for Trainium2, 8 NeuronCores.

Strategy
--------
Data-parallel over batch: each of the 8 cores runs B_local = 16 sequences
through the full T=784-step recurrence; outputs are concatenated on host.

Per-step math is algebraically folded into a single affine map.  With
u_t = e_x x_t + h_t e_h^T + m_t e_m^T, Ad = I + AT, w = W_m @ BT:

    m_{t+1} = (Ad + BT e_m) m_t + (BT e_h) h_t + (BT e_x) x_t
    pre_h   = (W_h + w e_h) h_t + (W_m Ad + w e_m) m_t + (W_x + w e_x) x_t
    h_{t+1} = tanh(pre_h)

so each step is one matmul  out[16, 1536] = s[16, 1537] @ M_big.T  plus a
tanh on the h columns.  M_big is precomputed on host in float64 -> fp16.

On-chip schedule (v2: DVE block-transpose).  The per-core batch is 16, so
each matmul's stationary operand (a state K-tile, [128, 16]) occupies 16
of the PE array's columns; the 1536 output features are split into 4
strips of 384 issued to the four 32-column groups (tile_position=(0,32j)),
so the 4 weight streams run concurrently: round pitch ~163ns = 384 cols
@ 2.4 GHz, stream-bound.  12 K-rounds + a rank-1 x round per step.

The key v2 change: the psum->state transposition is done by the DVE's
32x32 block transpose (one instr per region) instead of 3 PE transposes
+ 2 casts + copy.  The feature permutation is chosen so block semantics
line up: psum[32j+b, 32k+q] -> st[32j+q, 32k+b], i.e. output slot
(strip j, col 32k+q) IS state K-tile k, partition 32j+q.  Since batch
(16) < 32, the transposed batch lands in cols 32k..32k+16 and the slack
cols 32k+16..32k+32 hold transposed psum gap rows (kept zero by a
one-time psum memset; the x matmuls only write partitions 32j..32j+16).

Per step:  PE: x-round (4 concurrent K=1 rank-1 mms) + 4 m-rounds +
8 h-rounds;  ACT: tanh psum[:,0:256] -> hs (fp16);  DVE: T1 transpose
psum[:,256:384] -> st_m (unblocks m-rounds early), T2 transpose hs ->
st_h.  Issue order puts m-rounds before h-rounds so the T1/ACT/T2 chain
latency hides behind the x+m rounds.

x is stored compactly ([1, 16] per step on partition 0) and injected via
4 concurrent K=1 matmuls (lhsT = x_t at cols 0:16, rhs = strip j's xw),
whose start=True also clears the psum accumulation.  Two separate
half-body x tiles (A/B) are DMA-double-buffered so the per-body refill
never stalls the PE (the v1 single-tile WAR/RAW chain cost ~6.8us of
HAM-cold PE time per 56-step body).
"""

import numpy as np

import concourse.bass as bass
import concourse.mybir as mybir
import concourse.tile as tile
from concourse import bacc
from concourse.bass import ds, ts
from concourse.bass_utils import run_bass_kernel_spmd

T, B, UNITS, ORDER = 784, 128, 1024, 512
NCORES = 8
BL = B // NCORES          # 16 sequences per core
SOUT = UNITS + ORDER      # 1536 output features [pre_h; m']
KT = 12                   # state K-tiles of 128
NSTRIP = 4                # PE column groups
NW = SOUT // NSTRIP       # 384 output columns per strip
UNROLL = 112              # steps per For_i body (two x half-tiles of 56)
XH = UNROLL // 2          # steps per x half-tile

FP16 = mybir.dt.float16
FP32 = mybir.dt.float32


def _perms():
    # OP: psum slot (strip j, col c) -> feature
    OP = np.empty(SOUT, np.int64)
    for j in range(NSTRIP):
        for c in range(NW):
            OP[j * NW + c] = 256 * j + c if c < 256 else 1024 + 128 * j + (c - 256)
    # IP: (K-tile r, partition 32j+q) -> feature; consistent with the DVE
    # 32x32 block transpose of the psum laid out by OP.
    IP = np.empty(SOUT, np.int64)
    for r in range(KT):
        for j in range(NSTRIP):
            for q in range(32):
                IP[r * 128 + 32 * j + q] = (
                    256 * j + 32 * r + q if r < 8 else 1024 + 128 * j + 32 * (r - 8) + q
                )
    return OP, IP


def _build_weights(e_x, e_h, e_m, W_x, W_h, W_m, AT, BT):
    """Host-side fold into M_big (float64), then permute rows/cols."""
    f = np.float64
    e_x, e_h, e_m = e_x.astype(f), e_h.astype(f), e_m.astype(f)
    W_x, W_h, W_m = W_x.astype(f), W_h.astype(f), W_m.astype(f)
    AT, BT = AT.astype(f), BT.astype(f)
    Ad = np.eye(ORDER) + AT
    w = W_m @ BT                                   # (U, 1)
    top = np.concatenate([W_h + w @ e_h, W_m @ Ad + w @ e_m, W_x + w * e_x], axis=1)
    bot = np.concatenate([BT @ e_h, Ad + BT @ e_m, BT * e_x], axis=1)
    M_big = np.concatenate([top, bot], axis=0)     # (1536 out, 1537 in)
    OP, IP = _perms()
    Wstate = M_big[np.ix_(OP, IP)].T               # (in-perm, out-slot)
    xw = M_big[OP, SOUT]                           # x column, out-slot order
    return Wstate.astype(np.float16), xw.reshape(1, SOUT).astype(np.float16)


def _build_nc(t_steps=T, unroll=UNROLL):
    assert t_steps % unroll == 0 and unroll % 2 == 0
    iters = t_steps // unroll
    xh = unroll // 2
    nc = bacc.Bacc("TRN2", target_bir_lowering=False, num_devices=NCORES)

    w_dram = nc.dram_tensor("wts", [KT * 128, SOUT], FP16, kind="ExternalInput")
    xw_dram = nc.dram_tensor("xw", [1, SOUT], FP16, kind="ExternalInput")
    wd_dram = nc.dram_tensor("wd", [UNITS + 1, 10], FP16, kind="ExternalInput")
    # one body of zero padding at the end: body i prefetches slice i+1
    x_dram = nc.dram_tensor(
        "xs", [1, (t_steps + unroll) * BL], FP16, kind="ExternalInput"
    )
    out_dram = nc.dram_tensor("out", [BL, 10], FP32, kind="ExternalOutput")

    TANH = mybir.ActivationFunctionType.Tanh

    with tile.TileContext(nc) as tc:
        with (
            tc.tile_pool(name="const", bufs=1) as cpool,
            tc.tile_pool(name="state", bufs=1) as spool,
            tc.tile_pool(name="work", bufs=2) as wpool,
            tc.tile_pool(name="psum", bufs=1, space="PSUM") as ppool,
        ):
            # ---- persistent SBUF ----
            w_sb = cpool.tile([128, KT * SOUT], FP16, tag="w_sb")
            for r in range(KT):
                nc.sync.dma_start(w_sb[:, ts(r, SOUT)], w_dram[ts(r, 128), :])
            xw_sb = cpool.tile([1, SOUT], FP16, tag="xw_sb")
            nc.sync.dma_start(xw_sb[:, :], xw_dram[:, :])
            wd_sb = cpool.tile([128, 8 * 10], FP16, tag="wd_sb")
            for r in range(8):
                nc.sync.dma_start(wd_sb[:, ts(r, 10)], wd_dram[ts(r, 128), :])
            bias_sb = cpool.tile([1, 10], FP16, tag="bias_sb")
            nc.sync.dma_start(bias_sb[:, :], wd_dram[1024:1025, :])
            ones_sb = cpool.tile([1, BL], FP16, tag="ones_sb")
            nc.vector.memset(ones_sb[:, :], 1.0)

            # two separate x half-tiles: separate dep tracking, so the
            # prefetch of half A never gates the reads of half B
            x_stg = [
                spool.tile([1, xh * BL], FP16, tag=f"x_stg{a}", name=f"x_stg{a}")
                for a in range(2)
            ]
            nc.sync.dma_start(x_stg[0][:, :], x_dram[:, 0 : xh * BL])
            nc.sync.dma_start(x_stg[1][:, :], x_dram[:, xh * BL : unroll * BL])

            # state double buffers: rounds(t) read parity t%2, the
            # finalize of step t writes parity (t+1)%2
            stH = [
                spool.tile([128, 256], FP16, tag=f"stH{p}", name=f"stH{p}")
                for p in range(2)
            ]
            stM = [
                spool.tile([128, 128], FP16, tag=f"stM{p}", name=f"stM{p}")
                for p in range(2)
            ]
            msb = [
                spool.tile([128, 128], FP16, tag=f"ms{p}", name=f"ms{p}")
                for p in range(2)
            ]
            hsb = [
                spool.tile([128, 256], FP16, tag=f"hs{p}", name=f"hs{p}")
                for p in range(2)
            ]
            for t_ in stH + stM:
                nc.vector.memset(t_[:, :], 0.0)

            ps = [
                ppool.tile([128, NW], FP32, tag=f"ps{p}", name=f"ps{p}")
                for p in range(2)
            ]
            # zero once: partitions 32j+16..32j+32 are never written by any
            # matmul (x rounds are M=16), and the DVE transpose reads them
            # into the (never-read) slack cols of the state tiles
            for t_ in ps:
                nc.vector.memset(t_[:, :], 0.0)

            def step(u):
                p = u % 2
                wp = 1 - p
                psu = ps[p]
                xs = x_stg[0] if u < xh else x_stg[1]
                xoff = (u % xh) * BL
                # rank-1 x round: 4 concurrent K=1 mms, start=True clears
                # the batch partitions of all 4 strips
                for j in range(NSTRIP):
                    nc.tensor.matmul(
                        psu[ds(32 * j, BL), :],
                        xs[0:1, ds(xoff, BL)],
                        xw_sb[0:1, ds(j * NW, NW)],
                        start=True,
                        stop=False,
                        tile_position=(0, 32 * j),
                    )
                # m rounds first: they only need the previous step's
                # m-chain (CAST + T1), which is shorter than the h-chain
                for r in range(8, KT):
                    lhsT = stM[p][:, ds(32 * (r - 8), BL)]
                    for j in range(NSTRIP):
                        nc.tensor.matmul(
                            psu[ds(32 * j, BL), :],
                            lhsT,
                            w_sb[:, ds(r * SOUT + j * NW, NW)],
                            start=False,
                            stop=False,
                            tile_position=(0, 32 * j),
                        )
                # h rounds: gated on the previous step's tanh via stH
                for r in range(8):
                    lhsT = stH[p][:, ds(32 * r, BL)]
                    for j in range(NSTRIP):
                        nc.tensor.matmul(
                            psu[ds(32 * j, BL), :],
                            lhsT,
                            w_sb[:, ds(r * SOUT + j * NW, NW)],
                            start=False,
                            stop=(r == 7),
                            tile_position=(0, 32 * j),
                        )
                # finalize.  DVE: cast m-region to fp16, transpose it
                # (same-dtype requirement forbids a direct fp32->fp16
                # transpose), then transpose the tanh output.  ACT: tanh,
                # concurrent with the DVE m-chain.
                nc.vector.tensor_copy(msb[p][:, :], psu[:, 256:384])
                nc.vector.transpose(stM[wp][:, :], msb[p][:, :])
                nc.scalar.activation(hsb[p][:, :], psu[:, 0:256], TANH)
                nc.vector.transpose(stH[wp][:, :], hsb[p][:, :])

            with tc.For_i(0, iters, hint_engines=(mybir.EngineType.PE,)) as i:
                for u in range(unroll):
                    step(u)
                    if u == xh - 1:
                        # half A fully read; prefetch next body's half A
                        nc.sync.dma_start(
                            x_stg[0][:, :],
                            x_dram[:, ds((i + 1) * (unroll * BL), xh * BL)],
                        )
                nc.sync.dma_start(
                    x_stg[1][:, :],
                    x_dram[:, ds((i + 1) * (unroll * BL) + xh * BL, xh * BL)],
                )

            # ---- epilogue: logits = h W_d^T + b ; softmax ----
            # last step writes parity (783+1)%2 == 0
            ps_l = ps[0][0:BL, 0:10]   # reuse ps0's bank; loop is done
            for kt in range(8):
                nc.tensor.matmul(
                    ps_l,
                    stH[0][:, ds(32 * kt, BL)],
                    wd_sb[:, ts(kt, 10)],
                    start=(kt == 0),
                    stop=False,
                )
            nc.tensor.matmul(
                ps_l, ones_sb[:, :], bias_sb[:, :], start=False, stop=True
            )
            sm = wpool.tile([BL, 10], FP32, tag="sm")
            nc.scalar.activation(sm[:, :], ps_l, mybir.ActivationFunctionType.Exp)
            ssum = wpool.tile([BL, 1], FP32, tag="ssum")
            nc.vector.reduce_sum(ssum[:, :], sm[:, :], axis=mybir.AxisListType.X)
            srec = wpool.tile([BL, 1], FP32, tag="srec")
            nc.vector.reciprocal(srec[:, :], ssum[:, :])
            nc.vector.tensor_scalar_mul(sm[:, :], sm[:, :], srec[:, :])
            nc.sync.dma_start(out_dram[:, :], sm[:, :])

    nc.compile()
    return nc


_NC_CACHE = {}


def _get_nc(t_steps=T, unroll=UNROLL):
    key = (t_steps, unroll)
    if key not in _NC_CACHE:
        _NC_CACHE[key] = _build_nc(t_steps, unroll)
    return _NC_CACHE[key]


def kernel(inputs, e_x, e_h, e_m, W_x, W_h, W_m, AT, BT, W_dense, b_dense,
           _t_steps=T, _unroll=UNROLL, _trace=False):
    inputs = np.asarray(inputs, np.float32)
    args = [np.asarray(a, np.float32)
            for a in (e_x, e_h, e_m, W_x, W_h, W_m, AT, BT, W_dense, b_dense)]
    e_x, e_h, e_m, W_x, W_h, W_m, AT, BT, W_dense, b_dense = args

    wts, xw = _build_weights(e_x, e_h, e_m, W_x, W_h, W_m, AT, BT)
    _, IP = _perms()
    wd = np.zeros((UNITS + 1, 10), np.float16)
    wd[:UNITS, :] = W_dense.T[IP[:UNITS], :].astype(np.float16)
    wd[UNITS, :] = b_dense.astype(np.float16)

    x = inputs[:_t_steps, :, 0].astype(np.float16)        # (T, B)
    nc = _get_nc(_t_steps, _unroll)
    in_maps = []
    for c in range(NCORES):
        xc = x[:, c * BL:(c + 1) * BL]                    # (T, BL)
        xs = np.zeros((1, (_t_steps + _unroll) * BL), np.float16)
        xs[0, : _t_steps * BL] = np.ascontiguousarray(xc).reshape(-1)
        in_maps.append({"wts": wts, "xw": xw, "wd": wd, "xs": xs})

    res = run_bass_kernel_spmd(
        nc, in_maps, core_ids=list(range(NCORES)), trace=_trace
    )
    out = np.concatenate([res.results[c]["out"] for c in range(NCORES)], axis=0)
    kernel.last_results = res
    return out.astype(np.float32)


# revision 5
# speedup vs baseline: 1.3383x; 1.0935x over previous
"""LMU (Legendre Memory Unit) RNN kernel for Trainium2, 8 NeuronCores.

Strategy
--------
Data-parallel over batch: each of the 8 cores runs B_local = 16 sequences
through the full T=784-step recurrence; outputs are concatenated on host.

Per-step math is algebraically folded (host, float64).  With
u_t = e_x x_t + h_t e_h^T + m_t e_m^T, Ad = I + AT, w = W_m @ BT:

    m_{t+1} = Amm m_t + BT a_t + wx_m x_t,   a_t := e_h . h_t  (scalar/seq)
    pre_h   = Wq h_t + Wp m_t + wx_h x_t,    h_{t+1} = tanh(pre_h)

where Amm = Ad + BT e_m, Wq = W_h + w e_h, Wp = W_m Ad + w e_m.  The
m<-h coupling (BT e_h) is RANK-1, so h-rounds never write the m psum:
they emit only the 1024 pre_h columns plus one extra column a_t, and the
BT a_t term is injected one step later as a rank-1 "a-mm" using the
stored state m' := m - BT a_prev (compensations (Wp BT) a_prev and
(Amm BT) a_prev are folded into the a-mms; exact algebra, see
check_math.py).

On-chip schedule.  Per-core batch 16 -> state K-tiles [128, 16]
stationary, weights streamed, 4 column groups (tile_position=(0,32j))
concurrent.  Two psum tiles: psh [128, 257+] (pre_h + a col; written by
x/a/m/h rounds) and psm [128, 128] (m'; written by x/a/m rounds ONLY,
so it completes ~900ns before the step ends and its CAST+transpose
chain hides under the h-rounds).  Separate tiles also decouple the
tile-framework's reader chaining (ACT on psh vs CAST on psm).

The psum->state transposition is the DVE 32x32 block transpose; the
feature permutation makes block semantics line up: psum[32j+b, 32k+q] ->
st[32j+q, 32k+b], i.e. psum slot (strip j, col 32k+q) IS state K-tile k,
partition 32j+q.  Batch (16) < 32 so transposed batch occupies cols
32k..32k+16; slack cols hold transposed psum gap rows (zeros: one-time
psum memset; matmuls only write partitions 32j..32j+16).

Per step:  PE: x-mms (K=1 rank-1, start=True) -> m-rounds (4, fed by
stM) -> a-mms (K=1, fed by a_sb) -> h-rounds (8 x N=257, fed by stH).
DVE: CAST psm->fp16, T1 transpose -> stM; CAST_a psh a-block -> fp16,
T_a transpose -> a_sb; T2 transpose tanh output -> stH.
ACT: tanh psh[:,0:256] -> hs (issued after CAST_a: psh readers chain in
issue order).

x is stored compactly ([1, 16] per step on partition 0); two half-body
x tiles are DMA-double-buffered so the per-body refill never stalls.
"""

import numpy as np

import concourse.bass as bass
import concourse.mybir as mybir
import concourse.tile as tile
from concourse import bacc
from concourse.bass import ds, ts
from concourse.bass_utils import run_bass_kernel_spmd

T, B, UNITS, ORDER = 784, 128, 1024, 512
NCORES = 8
BL = B // NCORES          # 16 sequences per core
SOUT = UNITS + ORDER
KT = 12                   # state K-tiles of 128 (8 h + 4 m)
NSTRIP = 4                # PE column groups
NWH = 257                 # psh cols per strip: 256 pre_h + 1 a (strip 0)
NWM = 128                 # psm cols per strip
PSHW = 288                # psh tile width (a-block padded to 32)
UNROLL = 112              # steps per For_i body (two x half-tiles)
XH = UNROLL // 2

FP16 = mybir.dt.float16
FP32 = mybir.dt.float32


def _perms():
    # psum slot -> feature: psh (j, c<256) = h feature 256j+c;
    # psm (j, c) = m feature 128j+c (+1024 globally)
    # state: (K-tile r, partition 32j+q) -> feature
    IP = np.empty(SOUT, np.int64)
    for r in range(KT):
        for j in range(NSTRIP):
            for q in range(32):
                IP[r * 128 + 32 * j + q] = (
                    256 * j + 32 * r + q if r < 8 else 1024 + 128 * j + 32 * (r - 8) + q
                )
    return IP


def _build_weights(e_x, e_h, e_m, W_x, W_h, W_m, AT, BT):
    """Host-side fold into the phase-2' arrays (float64 -> fp16)."""
    f = np.float64
    e_x, e_h, e_m = e_x.astype(f), e_h.astype(f), e_m.astype(f)
    W_x, W_h, W_m = W_x.astype(f), W_h.astype(f), W_m.astype(f)
    AT, BT = AT.astype(f), BT.astype(f)
    Ad = np.eye(ORDER) + AT
    w = W_m @ BT                                    # (U, 1)
    Wq = W_h + w @ e_h                              # (U, U)
    Wp = W_m @ Ad + w @ e_m                         # (U, O)
    wx_h = (W_x + w * e_x)[:, 0]                    # (U,)
    Amm = Ad + BT @ e_m                             # (O, O)
    wx_m = (BT * e_x)[:, 0]                         # (O,)
    v_h = (Wp @ BT)[:, 0]                           # (U,) a-comp -> pre_h
    v_m = (Amm @ BT)[:, 0]                          # (O,) a-comp -> m'

    IP = _perms()
    IPh = IP[: 8 * 128]
    IPm = IP[8 * 128 :] - 1024
    OPh = [256 * j + np.arange(256) for j in range(NSTRIP)]
    OPm = [128 * j + np.arange(128) for j in range(NSTRIP)]

    wh = np.zeros((8 * 128, NSTRIP * NWH))
    wmh = np.zeros((4 * 128, NSTRIP * NWH))
    wmm = np.zeros((4 * 128, NSTRIP * NWM))
    xwh = np.zeros((1, NSTRIP * NWH))
    xwm = np.zeros((1, NSTRIP * NWM))
    awh = np.zeros((1, NSTRIP * NWH))
    awm = np.zeros((1, NSTRIP * NWM))
    for j in range(NSTRIP):
        ch = slice(j * NWH, j * NWH + 256)
        cm = slice(j * NWM, (j + 1) * NWM)
        wh[:, ch] = Wq[OPh[j]][:, IPh].T
        if j == 0:
            wh[:, j * NWH + 256] = e_h[0, IPh]
        wmh[:, ch] = Wp[OPh[j]][:, IPm].T
        wmm[:, cm] = Amm[OPm[j]][:, IPm].T
        xwh[0, ch] = wx_h[OPh[j]]
        xwm[0, cm] = wx_m[OPm[j]]
        awh[0, ch] = v_h[OPh[j]]
        awm[0, cm] = v_m[OPm[j]]
    h16 = np.float16
    return (wh.astype(h16), wmh.astype(h16), wmm.astype(h16),
            xwh.astype(h16), xwm.astype(h16), awh.astype(h16),
            awm.astype(h16))


def _build_nc(t_steps=T, unroll=UNROLL):
    assert t_steps % unroll == 0 and unroll % 2 == 0
    iters = t_steps // unroll
    xh = unroll // 2
    nc = bacc.Bacc("TRN2", target_bir_lowering=False, num_devices=NCORES)

    wh_dram = nc.dram_tensor("wh", [8 * 128, NSTRIP * NWH], FP16,
                             kind="ExternalInput")
    wmh_dram = nc.dram_tensor("wmh", [4 * 128, NSTRIP * NWH], FP16,
                              kind="ExternalInput")
    wmm_dram = nc.dram_tensor("wmm", [4 * 128, NSTRIP * NWM], FP16,
                              kind="ExternalInput")
    xwh_dram = nc.dram_tensor("xwh", [1, NSTRIP * NWH], FP16,
                              kind="ExternalInput")
    xwm_dram = nc.dram_tensor("xwm", [1, NSTRIP * NWM], FP16,
                              kind="ExternalInput")
    awh_dram = nc.dram_tensor("awh", [1, NSTRIP * NWH], FP16,
                              kind="ExternalInput")
    awm_dram = nc.dram_tensor("awm", [1, NSTRIP * NWM], FP16,
                              kind="ExternalInput")
    wd_dram = nc.dram_tensor("wd", [UNITS + 1, 10], FP16, kind="ExternalInput")
    x_dram = nc.dram_tensor(
        "xs", [1, (t_steps + unroll) * BL], FP16, kind="ExternalInput"
    )
    out_dram = nc.dram_tensor("out", [BL, 10], FP32, kind="ExternalOutput")

    TANH = mybir.ActivationFunctionType.Tanh

    with tile.TileContext(nc) as tc:
        with (
            tc.tile_pool(name="const", bufs=1) as cpool,
            tc.tile_pool(name="state", bufs=1) as spool,
            tc.tile_pool(name="work", bufs=2) as wpool,
            tc.tile_pool(name="psum", bufs=1, space="PSUM") as ppool,
        ):
            # ---- persistent SBUF ----
            wh_sb = cpool.tile([128, 8 * NSTRIP * NWH], FP16, tag="wh_sb")
            for r in range(8):
                nc.sync.dma_start(
                    wh_sb[:, ts(r, NSTRIP * NWH)], wh_dram[ts(r, 128), :]
                )
            wmh_sb = cpool.tile([128, 4 * NSTRIP * NWH], FP16, tag="wmh_sb")
            wmm_sb = cpool.tile([128, 4 * NSTRIP * NWM], FP16, tag="wmm_sb")
            for r in range(4):
                nc.sync.dma_start(
                    wmh_sb[:, ts(r, NSTRIP * NWH)], wmh_dram[ts(r, 128), :]
                )
                nc.sync.dma_start(
                    wmm_sb[:, ts(r, NSTRIP * NWM)], wmm_dram[ts(r, 128), :]
                )
            xwh_sb = cpool.tile([1, NSTRIP * NWH], FP16, tag="xwh_sb")
            xwm_sb = cpool.tile([1, NSTRIP * NWM], FP16, tag="xwm_sb")
            awh_sb = cpool.tile([1, NSTRIP * NWH], FP16, tag="awh_sb")
            awm_sb = cpool.tile([1, NSTRIP * NWM], FP16, tag="awm_sb")
            nc.sync.dma_start(xwh_sb[:, :], xwh_dram[:, :])
            nc.sync.dma_start(xwm_sb[:, :], xwm_dram[:, :])
            nc.sync.dma_start(awh_sb[:, :], awh_dram[:, :])
            nc.sync.dma_start(awm_sb[:, :], awm_dram[:, :])
            wd_sb = cpool.tile([128, 8 * 10], FP16, tag="wd_sb")
            for r in range(8):
                nc.sync.dma_start(wd_sb[:, ts(r, 10)], wd_dram[ts(r, 128), :])
            bias_sb = cpool.tile([1, 10], FP16, tag="bias_sb")
            nc.sync.dma_start(bias_sb[:, :], wd_dram[1024:1025, :])
            ones_sb = cpool.tile([1, BL], FP16, tag="ones_sb")
            nc.vector.memset(ones_sb[:, :], 1.0)

            x_stg = [
                spool.tile([1, xh * BL], FP16, tag=f"x_stg{a}", name=f"x_stg{a}")
                for a in range(2)
            ]
            nc.sync.dma_start(x_stg[0][:, :], x_dram[:, 0 : xh * BL])
            nc.sync.dma_start(x_stg[1][:, :], x_dram[:, xh * BL : unroll * BL])

            # state double buffers: step u reads parity u%2, writes 1-u%2
            stH = [
                spool.tile([128, 256], FP16, tag=f"stH{p}", name=f"stH{p}")
                for p in range(2)
            ]
            stM = [
                spool.tile([128, 128], FP16, tag=f"stM{p}", name=f"stM{p}")
                for p in range(2)
            ]
            a_sb = [
                spool.tile([32, 32], FP16, tag=f"a{p}", name=f"a{p}")
                for p in range(2)
            ]
            msb = [
                spool.tile([128, 128], FP16, tag=f"ms{p}", name=f"ms{p}")
                for p in range(2)
            ]
            asb16 = [
                spool.tile([32, 32], FP16, tag=f"as{p}", name=f"as{p}")
                for p in range(2)
            ]
            hsb = [
                spool.tile([128, 256], FP16, tag=f"hs{p}", name=f"hs{p}")
                for p in range(2)
            ]
            for t_ in stH + stM + a_sb:
                nc.vector.memset(t_[:, :], 0.0)

            psh = [
                ppool.tile([128, PSHW], FP32, tag=f"psh{p}", name=f"psh{p}")
                for p in range(2)
            ]
            psm = [
                ppool.tile([128, NWM], FP32, tag=f"psm{p}", name=f"psm{p}")
                for p in range(2)
            ]
            # zero once: gap partitions 32j+16..32j+32 (matmuls write only
            # 16 batch rows) and psh cols 257:288 feed the transposed slack
            for t_ in psh + psm:
                nc.vector.memset(t_[:, :], 0.0)

            def step(u):
                p = u % 2
                wp = 1 - p
                ph, pm = psh[p], psm[p]
                xs = x_stg[0] if u < xh else x_stg[1]
                xoff = (u % xh) * BL
                # rank-1 x rounds (start=True clears batch partitions)
                for j in range(NSTRIP):
                    xsl = xs[0:1, ds(xoff, BL)]
                    nc.tensor.matmul(
                        ph[ds(32 * j, BL), 0:NWH], xsl,
                        xwh_sb[0:1, ds(j * NWH, NWH)],
                        start=True, stop=False, tile_position=(0, 32 * j),
                    )
                    nc.tensor.matmul(
                        pm[ds(32 * j, BL), :], xsl,
                        xwm_sb[0:1, ds(j * NWM, NWM)],
                        start=True, stop=False, tile_position=(0, 32 * j),
                    )
                # m rounds (state m', 4 K-tiles) -> both psums
                for r in range(4):
                    lhsT = stM[p][:, ds(32 * r, BL)]
                    for j in range(NSTRIP):
                        nc.tensor.matmul(
                            ph[ds(32 * j, BL), 0:NWH], lhsT,
                            wmh_sb[:, ds(r * NSTRIP * NWH + j * NWH, NWH)],
                            start=False, stop=False, tile_position=(0, 32 * j),
                        )
                        nc.tensor.matmul(
                            pm[ds(32 * j, BL), :], lhsT,
                            wmm_sb[:, ds(r * NSTRIP * NWM + j * NWM, NWM)],
                            start=False, stop=False, tile_position=(0, 32 * j),
                        )
                # a-mms: rank-1 compensation with a(t-1); last psm writer
                for j in range(NSTRIP):
                    asl = a_sb[p][0:1, 0:BL]
                    nc.tensor.matmul(
                        ph[ds(32 * j, BL), 0:NWH], asl,
                        awh_sb[0:1, ds(j * NWH, NWH)],
                        start=False, stop=False, tile_position=(0, 32 * j),
                    )
                    nc.tensor.matmul(
                        pm[ds(32 * j, BL), :], asl,
                        awm_sb[0:1, ds(j * NWM, NWM)],
                        start=False, stop=True, tile_position=(0, 32 * j),
                    )
                # h rounds (8 K-tiles, N=257); last psh writer
                for r in range(8):
                    lhsT = stH[p][:, ds(32 * r, BL)]
                    for j in range(NSTRIP):
                        nc.tensor.matmul(
                            ph[ds(32 * j, BL), 0:NWH], lhsT,
                            wh_sb[:, ds(r * NSTRIP * NWH + j * NWH, NWH)],
                            start=False, stop=(r == 7),
                            tile_position=(0, 32 * j),
                        )
                # finalize.  DVE: m-chain first (psm completed early, its
                # chain hides under the h-rounds), then the a-block
                # (CAST_a issued before ACT: psh readers chain in issue
                # order), then T2 after tanh.
                nc.vector.tensor_copy(msb[p][:, :], pm[:, :])
                nc.vector.transpose(stM[wp][:, :], msb[p][:, :])
                nc.vector.tensor_copy(asb16[p][:, :], ph[0:32, 256:288])
                nc.vector.transpose(a_sb[wp][:, :], asb16[p][:, :])
                nc.scalar.activation(hsb[p][:, :], ph[:, 0:256], TANH)
                nc.vector.transpose(stH[wp][:, :], hsb[p][:, :])

            with tc.For_i(0, iters, hint_engines=(mybir.EngineType.PE,)) as i:
                for u in range(unroll):
                    step(u)
                    if u == xh - 1:
                        nc.sync.dma_start(
                            x_stg[0][:, :],
                            x_dram[:, ds((i + 1) * (unroll * BL), xh * BL)],
                        )
                nc.sync.dma_start(
                    x_stg[1][:, :],
                    x_dram[:, ds((i + 1) * (unroll * BL) + xh * BL, xh * BL)],
                )

            # ---- epilogue: logits = h W_d^T + b ; softmax ----
            ps_l = psh[0][0:BL, 0:10]   # loop done; reuse bank
            for kt in range(8):
                nc.tensor.matmul(
                    ps_l,
                    stH[0][:, ds(32 * kt, BL)],
                    wd_sb[:, ts(kt, 10)],
                    start=(kt == 0), stop=False,
                )
            nc.tensor.matmul(
                ps_l, ones_sb[:, :], bias_sb[:, :], start=False, stop=True
            )
            sm = wpool.tile([BL, 10], FP32, tag="sm")
            nc.scalar.activation(sm[:, :], ps_l, mybir.ActivationFunctionType.Exp)
            ssum = wpool.tile([BL, 1], FP32, tag="ssum")
            nc.vector.reduce_sum(ssum[:, :], sm[:, :], axis=mybir.AxisListType.X)
            srec = wpool.tile([BL, 1], FP32, tag="srec")
            nc.vector.reciprocal(srec[:, :], ssum[:, :])
            nc.vector.tensor_scalar_mul(sm[:, :], sm[:, :], srec[:, :])
            nc.sync.dma_start(out_dram[:, :], sm[:, :])

    nc.compile()
    return nc


_NC_CACHE = {}


def _get_nc(t_steps=T, unroll=UNROLL):
    key = (t_steps, unroll)
    if key not in _NC_CACHE:
        _NC_CACHE[key] = _build_nc(t_steps, unroll)
    return _NC_CACHE[key]


def kernel(inputs, e_x, e_h, e_m, W_x, W_h, W_m, AT, BT, W_dense, b_dense,
           _t_steps=T, _unroll=UNROLL, _trace=False):
    inputs = np.asarray(inputs, np.float32)
    args = [np.asarray(a, np.float32)
            for a in (e_x, e_h, e_m, W_x, W_h, W_m, AT, BT, W_dense, b_dense)]
    e_x, e_h, e_m, W_x, W_h, W_m, AT, BT, W_dense, b_dense = args

    wh, wmh, wmm, xwh, xwm, awh, awm = _build_weights(
        e_x, e_h, e_m, W_x, W_h, W_m, AT, BT
    )
    IP = _perms()
    wd = np.zeros((UNITS + 1, 10), np.float16)
    wd[:UNITS, :] = W_dense.T[IP[:UNITS], :].astype(np.float16)
    wd[UNITS, :] = b_dense.astype(np.float16)

    x = inputs[:_t_steps, :, 0].astype(np.float16)        # (T, B)
    nc = _get_nc(_t_steps, _unroll)
    in_maps = []
    for c in range(NCORES):
        xc = x[:, c * BL:(c + 1) * BL]                    # (T, BL)
        xs = np.zeros((1, (_t_steps + _unroll) * BL), np.float16)
        xs[0, : _t_steps * BL] = np.ascontiguousarray(xc).reshape(-1)
        in_maps.append({
            "wh": wh, "wmh": wmh, "wmm": wmm, "xwh": xwh, "xwm": xwm,
            "awh": awh, "awm": awm, "wd": wd, "xs": xs,
        })

    res = run_bass_kernel_spmd(
        nc, in_maps, core_ids=list(range(NCORES)), trace=_trace
    )
    out = np.concatenate([res.results[c]["out"] for c in range(NCORES)], axis=0)
    kernel.last_results = res
    return out.astype(np.float32)


# revision 7
# speedup vs baseline: 1.4341x; 1.0716x over previous
"""LMU (Legendre Memory Unit) RNN kernel for Trainium2, 8 NeuronCores.

Strategy
--------
Data-parallel over batch: each of the 8 cores runs B_local = 16 sequences
through the full T=784-step recurrence; outputs are concatenated on host.

Per-step math is algebraically folded (host, float64).  With
u_t = e_x x_t + h_t e_h^T + m_t e_m^T, Ad = I + AT, w = W_m @ BT:

    m_{t+1} = Amm m_t + BT a_t + wx_m x_t,   a_t := e_h . h_t  (scalar/seq)
    pre_h   = Wq h_t + Wp m_t + wx_h x_t,    h_{t+1} = tanh(pre_h)

where Amm = Ad + BT e_m, Wq = W_h + w e_h, Wp = W_m Ad + w e_m.  The
m<-h coupling (BT e_h) is RANK-1, so h-rounds never write the m psum:
they emit only the 1024 pre_h columns plus one extra column a_t, and the
BT a_t term is injected one step later as a rank-1 "a-mm" using the
stored state m' := m - BT a_prev (compensations (Wp BT) a_prev and
(Amm BT) a_prev are folded into the a-mms; exact algebra, see
check_math.py).

On-chip schedule.  Per-core batch 16 -> state K-tiles [128, 16]
stationary, weights streamed, 4 column groups (tile_position=(0,32j))
concurrent.  Two psum tiles: psh [128, 257+] (pre_h + a col; written by
x/a/m/h rounds) and psm [128, 128] (m'; written by x/a/m rounds ONLY,
so it completes ~900ns before the step ends and its CAST+transpose
chain hides under the h-rounds).  Separate tiles also decouple the
tile-framework's reader chaining (ACT on psh vs CAST on psm).

The psum->state transposition is the DVE 32x32 block transpose; the
feature permutation makes block semantics line up: psum[32j+b, 32k+q] ->
st[32j+q, 32k+b], i.e. psum slot (strip j, col 32k+q) IS state K-tile k,
partition 32j+q.  Batch (16) < 32 so transposed batch occupies cols
32k..32k+16; slack cols hold transposed psum gap rows (zeros: one-time
psum memset; matmuls only write partitions 32j..32j+16).

Per step:  PE: x-mms (K=1 rank-1, start=True) -> m-rounds (4, fed by
stM) -> a-mms (K=1, fed by a_sb) -> h-rounds (8 x N=257, fed by stH).
DVE: CAST psm->fp16, T1 transpose -> stM; CAST_a psh a-block -> fp16,
T_a transpose -> a_sb; T2 transpose tanh output -> stH.
ACT: tanh psh[:,0:256] -> hs (issued after CAST_a: psh readers chain in
issue order).

x is stored compactly ([1, 16] per step on partition 0); two half-body
x tiles are DMA-double-buffered so the per-body refill never stalls.
"""

import numpy as np

import concourse.bass as bass
import concourse.mybir as mybir
import concourse.tile as tile
from concourse import bacc
from concourse.bass import ds, ts
from concourse.bass_utils import run_bass_kernel_spmd

T, B, UNITS, ORDER = 784, 128, 1024, 512
NCORES = 8
BL = B // NCORES          # 16 sequences per core
SOUT = UNITS + ORDER
KT = 12                   # state K-tiles of 128 (8 h + 4 m)
NSTRIP = 4                # PE column groups
NWH = 257                 # psh cols per strip: 256 pre_h + 1 a (strip 0)
NWM = 128                 # psm cols per strip
PSHW = 288                # psh tile width (a-block padded to 32)
UNROLL = 112              # steps per For_i body (two x half-tiles)
XH = UNROLL // 2

FP16 = mybir.dt.float16
FP32 = mybir.dt.float32


def _perms():
    # psum slot -> feature: psh (j, c<256) = h feature 256j+c;
    # psm (j, c) = m feature 128j+c (+1024 globally)
    # state: (K-tile r, partition 32j+q) -> feature
    IP = np.empty(SOUT, np.int64)
    for r in range(KT):
        for j in range(NSTRIP):
            for q in range(32):
                IP[r * 128 + 32 * j + q] = (
                    256 * j + 32 * r + q if r < 8 else 1024 + 128 * j + 32 * (r - 8) + q
                )
    return IP


def _build_weights(e_x, e_h, e_m, W_x, W_h, W_m, AT, BT):
    """Host-side fold into the phase-2' arrays (float64 -> fp16)."""
    f = np.float64
    e_x, e_h, e_m = e_x.astype(f), e_h.astype(f), e_m.astype(f)
    W_x, W_h, W_m = W_x.astype(f), W_h.astype(f), W_m.astype(f)
    AT, BT = AT.astype(f), BT.astype(f)
    Ad = np.eye(ORDER) + AT
    w = W_m @ BT                                    # (U, 1)
    Wq = W_h + w @ e_h                              # (U, U)
    Wp = W_m @ Ad + w @ e_m                         # (U, O)
    wx_h = (W_x + w * e_x)[:, 0]                    # (U,)
    Amm = Ad + BT @ e_m                             # (O, O)
    wx_m = (BT * e_x)[:, 0]                         # (O,)
    v_h = (Wp @ BT)[:, 0]                           # (U,) a-comp -> pre_h
    v_m = (Amm @ BT)[:, 0]                          # (O,) a-comp -> m'

    IP = _perms()
    IPh = IP[: 8 * 128]
    IPm = IP[8 * 128 :] - 1024
    OPh = [256 * j + np.arange(256) for j in range(NSTRIP)]
    OPm = [128 * j + np.arange(128) for j in range(NSTRIP)]

    wh = np.zeros((8 * 128, NSTRIP * NWH))
    wmh = np.zeros((4 * 128, NSTRIP * NWH))
    wmm = np.zeros((4 * 128, NSTRIP * NWM))
    xwh = np.zeros((NSTRIP, NWH))
    xwm = np.zeros((NSTRIP, NWM))
    awh = np.zeros((1, NSTRIP * NWH))
    awm = np.zeros((1, NSTRIP * NWM))
    for j in range(NSTRIP):
        ch = slice(j * NWH, j * NWH + 256)
        cm = slice(j * NWM, (j + 1) * NWM)
        wh[:, ch] = Wq[OPh[j]][:, IPh].T
        if j == 0:
            wh[:, j * NWH + 256] = e_h[0, IPh]
        wmh[:, ch] = Wp[OPh[j]][:, IPm].T
        wmm[:, cm] = Amm[OPm[j]][:, IPm].T
        xwh[j, :256] = wx_h[OPh[j]]
        xwm[j, :] = wx_m[OPm[j]]
        awh[0, ch] = v_h[OPh[j]]
        awm[0, cm] = v_m[OPm[j]]
    h16 = np.float16
    return (wh.astype(h16), wmh.astype(h16), wmm.astype(h16),
            xwh.astype(h16), xwm.astype(h16), awh.astype(h16),
            awm.astype(h16))


def _build_nc(t_steps=T, unroll=UNROLL):
    assert t_steps % unroll == 0 and unroll % 2 == 0
    iters = t_steps // unroll
    xh = unroll // 2
    nc = bacc.Bacc("TRN2", target_bir_lowering=False, num_devices=NCORES)

    wh_dram = nc.dram_tensor("wh", [8 * 128, NSTRIP * NWH], FP16,
                             kind="ExternalInput")
    wmh_dram = nc.dram_tensor("wmh", [4 * 128, NSTRIP * NWH], FP16,
                              kind="ExternalInput")
    wmm_dram = nc.dram_tensor("wmm", [4 * 128, NSTRIP * NWM], FP16,
                              kind="ExternalInput")
    xwh_dram = nc.dram_tensor("xwh", [NSTRIP, NWH], FP16,
                              kind="ExternalInput")
    xwm_dram = nc.dram_tensor("xwm", [NSTRIP, NWM], FP16,
                              kind="ExternalInput")
    awh_dram = nc.dram_tensor("awh", [1, NSTRIP * NWH], FP16,
                              kind="ExternalInput")
    awm_dram = nc.dram_tensor("awm", [1, NSTRIP * NWM], FP16,
                              kind="ExternalInput")
    wd_dram = nc.dram_tensor("wd", [UNITS + 1, 10], FP16, kind="ExternalInput")
    x_dram = nc.dram_tensor(
        "xs", [NSTRIP, (t_steps + unroll) * 128], FP16, kind="ExternalInput"
    )
    out_dram = nc.dram_tensor("out", [BL, 10], FP32, kind="ExternalOutput")

    TANH = mybir.ActivationFunctionType.Tanh

    with tile.TileContext(nc) as tc:
        with (
            tc.tile_pool(name="const", bufs=1) as cpool,
            tc.tile_pool(name="state", bufs=1) as spool,
            tc.tile_pool(name="work", bufs=2) as wpool,
            tc.tile_pool(name="psum", bufs=1, space="PSUM") as ppool,
        ):
            # ---- persistent SBUF ----
            wh_sb = cpool.tile([128, 8 * NSTRIP * NWH], FP16, tag="wh_sb")
            for r in range(8):
                nc.sync.dma_start(
                    wh_sb[:, ts(r, NSTRIP * NWH)], wh_dram[ts(r, 128), :]
                )
            wmh_sb = cpool.tile([128, 4 * NSTRIP * NWH], FP16, tag="wmh_sb")
            wmm_sb = cpool.tile([128, 4 * NSTRIP * NWM], FP16, tag="wmm_sb")
            for r in range(4):
                nc.sync.dma_start(
                    wmh_sb[:, ts(r, NSTRIP * NWH)], wmh_dram[ts(r, 128), :]
                )
                nc.sync.dma_start(
                    wmm_sb[:, ts(r, NSTRIP * NWM)], wmm_dram[ts(r, 128), :]
                )
            xwh_sb = cpool.tile([NSTRIP, NWH], FP16, tag="xwh_sb")
            xwm_sb = cpool.tile([NSTRIP, NWM], FP16, tag="xwm_sb")
            awh_sb = cpool.tile([1, NSTRIP * NWH], FP16, tag="awh_sb")
            awm_sb = cpool.tile([1, NSTRIP * NWM], FP16, tag="awm_sb")
            nc.sync.dma_start(xwh_sb[:, :], xwh_dram[:, :])
            nc.sync.dma_start(xwm_sb[:, :], xwm_dram[:, :])
            nc.sync.dma_start(awh_sb[:, :], awh_dram[:, :])
            nc.sync.dma_start(awm_sb[:, :], awm_dram[:, :])
            wd_sb = cpool.tile([128, 8 * 10], FP16, tag="wd_sb")
            for r in range(8):
                nc.sync.dma_start(wd_sb[:, ts(r, 10)], wd_dram[ts(r, 128), :])
            bias_sb = cpool.tile([1, 10], FP16, tag="bias_sb")
            nc.sync.dma_start(bias_sb[:, :], wd_dram[1024:1025, :])
            ones_sb = cpool.tile([1, BL], FP16, tag="ones_sb")
            nc.vector.memset(ones_sb[:, :], 1.0)

            x_stg = [
                spool.tile(
                    [NSTRIP, xh * 128], FP16, tag=f"x_stg{a}", name=f"x_stg{a}"
                )
                for a in range(2)
            ]
            nc.sync.dma_start(x_stg[0][:, :], x_dram[:, 0 : xh * 128])
            nc.sync.dma_start(x_stg[1][:, :], x_dram[:, xh * 128 : unroll * 128])

            # state double buffers: step u reads parity u%2, writes 1-u%2
            stHa = [
                spool.tile([128, 128], FP16, tag=f"stHa{p}", name=f"stHa{p}")
                for p in range(2)
            ]
            stHb = [
                spool.tile([128, 128], FP16, tag=f"stHb{p}", name=f"stHb{p}")
                for p in range(2)
            ]
            stM = [
                spool.tile([128, 128], FP16, tag=f"stM{p}", name=f"stM{p}")
                for p in range(2)
            ]
            a_sb = [
                spool.tile([32, 32], FP16, tag=f"a{p}", name=f"a{p}")
                for p in range(2)
            ]
            msb = [
                spool.tile([128, 128], FP16, tag=f"ms{p}", name=f"ms{p}")
                for p in range(2)
            ]
            asb16 = [
                spool.tile([32, 32], FP16, tag=f"as{p}", name=f"as{p}")
                for p in range(2)
            ]
            hsa = [
                spool.tile([128, 128], FP16, tag=f"hsa{p}", name=f"hsa{p}")
                for p in range(2)
            ]
            hsb = [
                spool.tile([128, 128], FP16, tag=f"hsb{p}", name=f"hsb{p}")
                for p in range(2)
            ]
            for t_ in stHa + stHb + stM + a_sb:
                nc.vector.memset(t_[:, :], 0.0)

            psh = [
                ppool.tile([128, PSHW], FP32, tag=f"psh{p}", name=f"psh{p}")
                for p in range(2)
            ]
            psm = [
                ppool.tile([128, NWM], FP32, tag=f"psm{p}", name=f"psm{p}")
                for p in range(2)
            ]
            # zero once: gap partitions 32j+16..32j+32 (matmuls write only
            # 16 batch rows) and psh cols 257:288 feed the transposed slack
            for t_ in psh + psm:
                nc.vector.memset(t_[:, :], 0.0)

            def step(u):
                p = u % 2
                wp = 1 - p
                ph, pm = psh[p], psm[p]
                xs = x_stg[0] if u < xh else x_stg[1]
                xoff = (u % xh) * 128
                # x rounds: K=4 host-built spread (row k carries x at cols
                # 32k..32k+16, rhs row k = strip k weights); one mm per
                # psum tile, M=128 start=True clears gaps too
                xsl = xs[:, ds(xoff, 128)]
                nc.tensor.matmul(
                    ph[:, 0:NWH], xsl, xwh_sb[:, :],
                    start=True, stop=False,
                )
                nc.tensor.matmul(
                    pm[:, :], xsl, xwm_sb[:, :],
                    start=True, stop=False,
                )
                # m rounds (state m', 4 K-tiles) -> both psums
                for r in range(4):
                    lhsT = stM[p][:, ds(32 * r, BL)]
                    for j in range(NSTRIP):
                        nc.tensor.matmul(
                            ph[ds(32 * j, BL), 0:NWH], lhsT,
                            wmh_sb[:, ds(r * NSTRIP * NWH + j * NWH, NWH)],
                            start=False, stop=False, tile_position=(0, 32 * j),
                        )
                        nc.tensor.matmul(
                            pm[ds(32 * j, BL), :], lhsT,
                            wmm_sb[:, ds(r * NSTRIP * NWM + j * NWM, NWM)],
                            start=False, stop=False, tile_position=(0, 32 * j),
                        )
                # a-mms: rank-1 compensation with a(t-1); last psm writer
                for j in range(NSTRIP):
                    asl = a_sb[p][0:1, 0:BL]
                    nc.tensor.matmul(
                        ph[ds(32 * j, BL), 0:NWH], asl,
                        awh_sb[0:1, ds(j * NWH, NWH)],
                        start=False, stop=False, tile_position=(0, 32 * j),
                    )
                    nc.tensor.matmul(
                        pm[ds(32 * j, BL), :], asl,
                        awm_sb[0:1, ds(j * NWM, NWM)],
                        start=False, stop=True, tile_position=(0, 32 * j),
                    )
                # h rounds (8 K-tiles, N=257); last psh writer
                for r in range(8):
                    lhsT = (stHa[p][:, ds(32 * r, BL)] if r < 4
                            else stHb[p][:, ds(32 * (r - 4), BL)])
                    for j in range(NSTRIP):
                        nc.tensor.matmul(
                            ph[ds(32 * j, BL), 0:NWH], lhsT,
                            wh_sb[:, ds(r * NSTRIP * NWH + j * NWH, NWH)],
                            start=False, stop=(r == 7),
                            tile_position=(0, 32 * j),
                        )
                # finalize.  DVE: m-chain first (psm completed early, its
                # chain hides under the h-rounds), then the a-block
                # (CAST_a issued before ACT: psh readers chain in issue
                # order), then T2 after tanh.
                nc.vector.tensor_copy(msb[p][:, :], pm[:, :])
                nc.vector.transpose(stM[wp][:, :], msb[p][:, :])
                nc.vector.tensor_copy(asb16[p][:, :], ph[0:32, 256:288])
                nc.vector.transpose(a_sb[wp][:, :], asb16[p][:, :])
                nc.scalar.activation(hsa[p][:, :], ph[:, 0:128], TANH)
                nc.scalar.activation(hsb[p][:, :], ph[:, 128:256], TANH)
                nc.vector.transpose(stHa[wp][:, :], hsa[p][:, :])
                nc.vector.transpose(stHb[wp][:, :], hsb[p][:, :])

            with tc.For_i(0, iters, hint_engines=(mybir.EngineType.PE,)) as i:
                for u in range(unroll):
                    step(u)
                    if u == xh - 1:
                        nc.sync.dma_start(
                            x_stg[0][:, :],
                            x_dram[:, ds((i + 1) * (unroll * 128), xh * 128)],
                        )
                nc.sync.dma_start(
                    x_stg[1][:, :],
                    x_dram[:, ds((i + 1) * (unroll * 128) + xh * 128, xh * 128)],
                )

            # ---- epilogue: logits = h W_d^T + b ; softmax ----
            ps_l = psh[0][0:BL, 0:10]   # loop done; reuse bank
            for kt in range(8):
                lhsT = (stHa[0][:, ds(32 * kt, BL)] if kt < 4
                        else stHb[0][:, ds(32 * (kt - 4), BL)])
                nc.tensor.matmul(
                    ps_l, lhsT, wd_sb[:, ts(kt, 10)],
                    start=(kt == 0), stop=False,
                )
            nc.tensor.matmul(
                ps_l, ones_sb[:, :], bias_sb[:, :], start=False, stop=True
            )
            sm = wpool.tile([BL, 10], FP32, tag="sm")
            nc.scalar.activation(sm[:, :], ps_l, mybir.ActivationFunctionType.Exp)
            ssum = wpool.tile([BL, 1], FP32, tag="ssum")
            nc.vector.reduce_sum(ssum[:, :], sm[:, :], axis=mybir.AxisListType.X)
            srec = wpool.tile([BL, 1], FP32, tag="srec")
            nc.vector.reciprocal(srec[:, :], ssum[:, :])
            nc.vector.tensor_scalar_mul(sm[:, :], sm[:, :], srec[:, :])
            nc.sync.dma_start(out_dram[:, :], sm[:, :])

    nc.compile()
    return nc


_NC_CACHE = {}


def _get_nc(t_steps=T, unroll=UNROLL):
    key = (t_steps, unroll)
    if key not in _NC_CACHE:
        _NC_CACHE[key] = _build_nc(t_steps, unroll)
    return _NC_CACHE[key]


def kernel(inputs, e_x, e_h, e_m, W_x, W_h, W_m, AT, BT, W_dense, b_dense,
           _t_steps=T, _unroll=UNROLL, _trace=False):
    inputs = np.asarray(inputs, np.float32)
    args = [np.asarray(a, np.float32)
            for a in (e_x, e_h, e_m, W_x, W_h, W_m, AT, BT, W_dense, b_dense)]
    e_x, e_h, e_m, W_x, W_h, W_m, AT, BT, W_dense, b_dense = args

    wh, wmh, wmm, xwh, xwm, awh, awm = _build_weights(
        e_x, e_h, e_m, W_x, W_h, W_m, AT, BT
    )
    IP = _perms()
    wd = np.zeros((UNITS + 1, 10), np.float16)
    wd[:UNITS, :] = W_dense.T[IP[:UNITS], :].astype(np.float16)
    wd[UNITS, :] = b_dense.astype(np.float16)

    x = inputs[:_t_steps, :, 0].astype(np.float16)        # (T, B)
    nc = _get_nc(_t_steps, _unroll)
    in_maps = []
    for c in range(NCORES):
        xc = x[:, c * BL:(c + 1) * BL]                    # (T, BL)
        xs4 = np.zeros((NSTRIP, _t_steps + _unroll, 128), np.float16)
        for k in range(NSTRIP):
            xs4[k, :_t_steps, 32 * k : 32 * k + BL] = xc
        xs = np.ascontiguousarray(
            xs4.reshape(NSTRIP, (_t_steps + _unroll) * 128)
        )
        in_maps.append({
            "wh": wh, "wmh": wmh, "wmm": wmm, "xwh": xwh, "xwm": xwm,
            "awh": awh, "awm": awm, "wd": wd, "xs": xs,
        })

    res = run_bass_kernel_spmd(
        nc, in_maps, core_ids=list(range(NCORES)), trace=_trace
    )
    out = np.concatenate([res.results[c]["out"] for c in range(NCORES)], axis=0)
    kernel.last_results = res
    return out.astype(np.float32)
